# revision 21
# baseline (speedup 1.0000x reference)
"""CFConv (SchNet continuous-filter convolution) Bass/Tile kernel for 8x TRN2.

Reference computation (per molecule b):
    W   = ssp(f_ij @ fw1 + fb1) @ fw2 + fb2          (B,A,N,F); ssp = softplus - ln2
    C   = 0.5*(cos(r_ij*pi/5)+1) * (r_ij<5) * mask   (B,A,N)
    y   = x @ in2f_w                                  (B,A,F)
    out = sum_n y[b, nbr[b,a,n], :] * W * C[...,None] (B,A,F)

Sharding: data-parallel over batch B=32 across 8 cores (4 molecules/core).

Per-core device plan (rows = flattened (a,n), 65536 rows in 32 quad-groups of
2048; ACT is the bottleneck engine, ~1 elem/cycle/lane, so ssp costs 2 passes):
  MM1  (PE):  out1T[h,r]  = fw1.T @ f_ijT            4x N=512, feature-major
  ssp  (ACT): t = Exp(out1T + fb1); W1s = Ln(0.5t + 0.5) == softplus(v) - ln2
              (exact; Softplus LUT was overlaid away on this stack, Exp+Ln
              share one resident table set via _pin_act_tables)
  MM2  (PE):  out2[r,f]   = W1s_tile.T @ fw2         16x N=128, row-major
  gath (DMA): y_nbh rows from y_dram (dma_gather, 1024 idxs/instr max)
  mul  (DVE): P = out2_psum * y_nbh                  -> SBUF bf16
  agg  (PE):  outT[f, 2t:2t+2] = P_tile.T @ C_bd     cutoff C folded into C_bd
  epilogue:   PE-transpose outT (per 512-atom half) -> out rows, DMA out.
"""

import os
import sys
from contextlib import ExitStack

import numpy as np

# Prefer /opt/trn_rl_repo (its bass_rust maps pwp "softplus" -> Softplus).
for _p in ("/root/.axon_site/_ro/trn_rl_repo", "/opt/trn_rl_repo"):
    if os.path.isdir(_p) and _p not in sys.path:
        sys.path.insert(0, _p)

import ml_dtypes  # noqa: E402
import concourse.bass as bass  # noqa: E402
import concourse.tile as tile  # noqa: E402
from concourse import bacc, mybir  # noqa: E402
from concourse.bass_utils import run_bass_kernel_spmd  # noqa: E402

BF16 = mybir.dt.bfloat16
FP32 = mybir.dt.float32
I16 = mybir.dt.int16
AF = mybir.ActivationFunctionType
ALU = mybir.AluOpType

B, A, N, G, F = 32, 256, 64, 50, 128
CUTOFF = 5.0
NCORES = 8
BPC = B // NCORES              # molecules per core = 4
ROWS = BPC * A * N             # rows per core = 65536
GROUP = 512                    # rows per group (one PSUM bank)
NGROUPS = ROWS // GROUP        # 128
TPG = GROUP // 128             # 128-row tiles per group = 4
NTILES = ROWS // 128           # 512
ATOMS = BPC * A                # 1024 atoms per core
LN2 = float(np.log(2.0))

# every k-th group uses ACT psum-evac + DVE sbuf-multiply instead of a direct
# DVE psum-source multiply (engine load balance knob; tune on HW). 0 = off.
ACT_ASSIST_EVERY = int(os.environ.get("CF_ACT_ASSIST", "0"))
SSP_MODE = os.environ.get("CF_SSP_MODE", "exp_ln")  # exp_ln|softplus|sigmoid_ln
SUB_ENGINE = os.environ.get("CF_SUB_ENGINE", "pool")  # pool | dve (-ln2 shift)

_CACHE: dict = {}
LAST_RESULTS = None


def _bf16(x):
    return np.asarray(np.asarray(x, dtype=np.float32), dtype=ml_dtypes.bfloat16)


def _pin_act_tables():
    """Restrict the ACT table-set chooser to {natural_log_exp_and_others,
    trig_and_small} so Exp+Ln share one resident LUT set (the default greedy
    chooser alternates exp_and_others/natural_log -> ~83us of table reloads).
    Mutates the functools.cache'd dict in place; set ids stay aligned with
    act_info.json because only the *contents* are blanked."""
    from concourse.hw_specs import get_activation_tables
    keep = {"natural_log_exp_and_others", "trig_and_small"}
    tabs = get_activation_tables("gen3")
    if any(k in tabs for k in keep):
        for k in list(tabs.keys()):
            if k not in keep:
                tabs[k] = set()


def build_kernel(fb2_nonzero: bool, need_pmask: bool, ssp_mode: str = SSP_MODE):
    """Builds the Bass program (shared by all 8 cores)."""
    if ssp_mode == "exp_ln":
        _pin_act_tables()
    nc = bacc.Bacc("TRN2", target_bir_lowering=False, debug=False)

    # ---- DRAM I/O (per-core shards; fw2 is pre-negated in sigmoid_ln mode) ----
    d_fijT = nc.dram_tensor("fijT", [G, ROWS], BF16, kind="ExternalInput")
    d_r = nc.dram_tensor("r_ij", [128, ROWS // 128], FP32, kind="ExternalInput")
    d_xT = nc.dram_tensor("xT", [F, ATOMS], BF16, kind="ExternalInput")
    d_idx = nc.dram_tensor("idx", [128, ROWS // 16], I16, kind="ExternalInput")
    d_fw1 = nc.dram_tensor("fw1", [G, F], BF16, kind="ExternalInput")
    d_fw2 = nc.dram_tensor("fw2", [F, F], BF16, kind="ExternalInput")
    d_w2f = nc.dram_tensor("in2f_w", [F, F], BF16, kind="ExternalInput")
    d_fb1 = nc.dram_tensor("fb1", [F, 1], FP32, kind="ExternalInput")
    d_eye = nc.dram_tensor("eye", [128, 128], FP32, kind="ExternalInput")
    d_pm = nc.dram_tensor("pmask", [128, ROWS // 128], FP32, kind="ExternalInput")
    d_fb2r = nc.dram_tensor("fb2row", [1, GROUP], BF16, kind="ExternalInput")
    d_out = nc.dram_tensor("out", [ATOMS, F], FP32, kind="ExternalOutput")

    with tile.TileContext(nc) as tc, ExitStack() as ctx:
        consts = ctx.enter_context(tc.tile_pool(name="consts", bufs=1))
        w1pool = ctx.enter_context(tc.tile_pool(name="w1", bufs=int(os.environ.get("CF_W1BUFS", "3"))))
        ypool = ctx.enter_context(tc.tile_pool(name="ynbh", bufs=int(os.environ.get("CF_YBUFS", "4"))))
        ppool = ctx.enter_context(tc.tile_pool(name="pmul", bufs=int(os.environ.get("CF_PBUFS", "3"))))
        evpool = ctx.enter_context(tc.tile_pool(name="evac", bufs=3))
        fijpool = ctx.enter_context(tc.tile_pool(name="fij", bufs=int(os.environ.get("CF_FIJBUFS", "4"))))
        outsb = ctx.enter_context(tc.tile_pool(name="outsb", bufs=1))
        ps_mm1 = ctx.enter_context(tc.tile_pool(name="psmm1", bufs=1, space="PSUM"))
        ps_mm2 = ctx.enter_context(tc.tile_pool(name="psmm2", bufs=2, space="PSUM"))
        ps_acc = ctx.enter_context(tc.tile_pool(name="psacc", bufs=2, space="PSUM"))
        dram = ctx.enter_context(tc.tile_pool(name="dram", bufs=1, space="DRAM"))

        # ---- load constants (r_ij first: it gates the serial C-prologue) ----
        r_sb = consts.tile([128, ROWS // 128], FP32)
        nc.sync.dma_start(r_sb[:], d_r[:])
        fw1 = consts.tile([G, F], BF16)
        nc.sync.dma_start(fw1[:], d_fw1[:])
        fw2 = consts.tile([F, F], BF16)
        nc.sync.dma_start(fw2[:], d_fw2[:])
        w2f = consts.tile([F, F], BF16)
        nc.sync.dma_start(w2f[:], d_w2f[:])
        fb1 = consts.tile([F, 1], FP32)
        nc.sync.dma_start(fb1[:], d_fb1[:])
        eye = consts.tile([128, 128], FP32)
        nc.sync.dma_start(eye[:], d_eye[:])
        idxs = consts.tile([128, ROWS // 16], I16)
        nc.sync.dma_start(idxs[:], d_idx[:])
        if ssp_mode == "sigmoid_ln":
            fb1n = consts.tile([F, 1], FP32)
            nc.vector.tensor_scalar_mul(fb1n[:], fb1[:], -1.0)
            zerob = consts.tile([128, 1], FP32)
            nc.vector.memset(zerob[:], 0.0)
        if ssp_mode == "exp_ln":
            half = consts.tile([128, 1], FP32)
            nc.vector.memset(half[:], 0.5)
        if fb2_nonzero:
            fb2row = consts.tile([1, GROUP], BF16)
            nc.sync.dma_start(fb2row[:], d_fb2r[:])
            ones1 = consts.tile([1, 128], BF16)
            nc.vector.memset(ones1[:], 1.0)

        # ---- prologue: y = x @ in2f_w  -> y_dram (bf16) ----
        y_dram = dram.tile([ATOMS, F], BF16)
        xT = consts.tile([F, ATOMS], BF16)
        nc.sync.dma_start(xT[:], d_xT[:])
        y_sb = consts.tile([128, ATOMS // 128, F], BF16)
        for blk in range(ATOMS // 128):
            yps = ps_acc.tile([128, GROUP], FP32, tag="acc")
            nc.tensor.matmul(yps[:, 0:F], xT[:, bass.ts(blk, 128)], w2f[:],
                             start=True, stop=True)
            nc.vector.tensor_copy(y_sb[:, blk, :], yps[:, 0:F])
        nc.sync.dma_start(
            y_dram[:].rearrange("(b p) f -> p b f", p=128), y_sb[:])

        # ---- prologue: cutoff C -> block-diag C_bd [128, 2*NTILES] bf16 ----
        # cos(t) = sin(pi/2 - t); ACT Sin valid range is [-pi, pi].
        c_nat = consts.tile([128, ROWS // 128], FP32)
        sinb = consts.tile([128, 1], FP32)
        nc.vector.memset(sinb[:], float(np.pi / 2))
        nc.scalar.activation(c_nat[:], r_sb[:], AF.Sin,
                             bias=sinb[:], scale=float(-np.pi / CUTOFF))
        nc.vector.tensor_scalar(c_nat[:], c_nat[:], 0.5, 0.5, ALU.mult, ALU.add)
        rmask = consts.tile([128, ROWS // 128], FP32)
        nc.vector.tensor_scalar(rmask[:], r_sb[:], CUTOFF, None, ALU.is_lt)
        nc.vector.tensor_mul(c_nat[:], c_nat[:], rmask[:])
        if need_pmask:
            pm_sb = consts.tile([128, ROWS // 128], FP32)
            nc.sync.dma_start(pm_sb[:], d_pm[:])
            nc.vector.tensor_mul(c_nat[:], c_nat[:], pm_sb[:])
        # transpose 128-blocks:  c_T[:, 4j+b] = c_nat[:, 128b:128b+128].T[:, j]
        c_T = consts.tile([128, NTILES], FP32)
        nblk = NTILES // 128  # 4
        for b in range(nblk):
            tps = ps_acc.tile([128, GROUP], FP32, tag="acc")
            nc.tensor.transpose(tps[:, 0:128], c_nat[:, bass.ts(b, 128)], eye[:])
            nc.vector.tensor_copy(c_T[:, b::nblk], tps[:, 0:128])
        c_bd = consts.tile([128, 2 * NTILES], BF16)
        nc.vector.memset(c_bd[:], 0.0)
        nc.vector.tensor_copy(c_bd[0:64, 0::2], c_T[0:64, :])
        nc.vector.tensor_copy(c_bd[64:128, 1::2], c_T[64:128, :])

        # ---- main loop: quad-groups of 2048 rows (32 iterations) ----
        # mm1 psum is a single 4-bank [128, 2048] tile (bufs=1): ACT fixed
        # overheads amortize over FD=2048. MM1 for iteration k+1 is emitted
        # immediately after Exp(k) (software pipelining) so the PE refills the
        # mm1 bank while ACT runs Ln(k) -- otherwise Exp(k+1) stalls.
        outT_sb = outsb.tile([128, ATOMS], FP32)
        out_rows = outsb.tile([128, ATOMS // 128, F], FP32)
        QG = 4 * GROUP                  # 2048 rows per iteration
        NQG = ROWS // QG                # 32
        FILL = 256                      # tiles per acc-psum fill
        qgpf = FILL // (QG // 128)      # quad-groups per fill = 16
        acc = None

        GHALF = QG // 2  # 1024 idxs per dma_gather instruction
        def do_gather(g):
            ynbh = ypool.tile([128, QG // 128, F], BF16, tag="ynbh")
            for q in range(2):
                nc.gpsimd.dma_gather(
                    ynbh[:, q * (GHALF // 128):(q + 1) * (GHALF // 128), :],
                    y_dram[:],
                    idxs[:, bass.ts(2 * g + q, GHALF // 16)],
                    GHALF, GHALF, F)
            return ynbh

        def do_mm1(g):
            fij = fijpool.tile([G, QG], BF16, tag="fij")
            nc.sync.dma_start(fij[:], d_fijT[:, bass.ts(g, QG)])
            p1 = ps_mm1.tile([128, QG], FP32, tag="mm1")
            for h in range(4):
                nc.tensor.matmul(p1[:, bass.ts(h, GROUP)], fw1[:],
                                 fij[:, bass.ts(h, GROUP)],
                                 start=True, stop=True)
            return p1

        p1 = do_mm1(0)
        ynbh = do_gather(0)
        for g in range(NQG):
            if g % qgpf == 0:
                acc = ps_acc.tile([128, GROUP], FP32, tag="acc")

            # ssp: W1s = softplus(out1T + fb1) - ln2, in SBUF bf16
            w1s = w1pool.tile([128, QG], BF16, tag="w1s")
            if ssp_mode == "exp_ln":
                # t = exp(v + fb1); w1s = ln(0.5*t + 0.5) == ssp(v) exactly.
                ex = w1pool.tile([128, QG], BF16, tag="w1")
                nc.scalar.activation(ex[:], p1[:], AF.Exp, bias=fb1[:])
                if g + 1 < NQG:
                    p1 = do_mm1(g + 1)
                nc.scalar.activation(w1s[:], ex[:], AF.Ln, bias=half[:],
                                     scale=0.5)
            elif ssp_mode == "softplus":
                w1 = w1pool.tile([128, QG], BF16, tag="w1")
                nc.scalar.activation(w1[:], p1[:], AF.Softplus, bias=fb1[:])
                if g + 1 < NQG:
                    p1 = do_mm1(g + 1)
                if SUB_ENGINE == "pool":
                    nc.gpsimd.tensor_scalar_sub(w1s[:], w1[:], LN2)
                else:
                    nc.vector.tensor_scalar_sub(w1s[:], w1[:], LN2)
            else:
                # w1s = ln(2*sigmoid(-(v))) = -(ssp(v)); fw2 pre-negated on host.
                sg = w1pool.tile([128, QG], BF16, tag="w1")
                nc.scalar.activation(sg[:], p1[:], AF.Sigmoid,
                                     bias=fb1n[:], scale=-1.0)
                if g + 1 < NQG:
                    p1 = do_mm1(g + 1)
                nc.scalar.activation(w1s[:], sg[:], AF.Ln, bias=zerob[:],
                                     scale=2.0)

            ynbh_cur = ynbh
            if g + 1 < NQG:
                ynbh = do_gather(g + 1)

            # per 512-row half: MM2, multiply, aggregate
            for hh in range(4):
                p2 = ps_mm2.tile([128, GROUP], FP32, tag="mm2")
                for t in range(TPG):
                    nc.tensor.matmul(
                        p2[:, bass.ts(t, F)],
                        w1s[:, bass.ts(hh * TPG + t, 128)], fw2[:],
                        start=True, stop=(not fb2_nonzero))
                if fb2_nonzero:
                    nc.tensor.matmul(p2[:], ones1[:], fb2row[:],
                                     start=False, stop=True,
                                     skip_group_check=True)

                yg = ynbh_cur[:, hh * TPG:(hh + 1) * TPG, :]
                psb = ppool.tile([128, TPG, F], BF16, tag="p")
                if ACT_ASSIST_EVERY and (4 * g + hh) % ACT_ASSIST_EVERY == 0:
                    ev = evpool.tile([128, GROUP], BF16, tag="ev")
                    nc.scalar.copy(ev[:], p2[:])
                    nc.vector.tensor_mul(
                        psb[:].rearrange("p t f -> p (t f)"), ev[:],
                        yg.rearrange("p t f -> p (t f)"))
                else:
                    nc.vector.tensor_mul(
                        psb[:].rearrange("p t f -> p (t f)"), p2[:],
                        yg.rearrange("p t f -> p (t f)"))

                # agg: outT[:, 2tau:2tau+2] = P_tile.T @ C_bd[:, 2tau:2tau+2]
                for t in range(TPG):
                    tau = (4 * g + hh) * TPG + t
                    col = (tau % FILL) * 2
                    nc.tensor.matmul(acc[:, col:col + 2], psb[:, t, :],
                                     c_bd[:, 2 * tau:2 * tau + 2],
                                     start=True, stop=True)

            if g % qgpf == qgpf - 1:
                hseg = g // qgpf
                nc.vector.tensor_copy(outT_sb[:, bass.ts(hseg, GROUP)], acc[:])
                for blk in range(4 * hseg, 4 * hseg + 4):
                    tps = ps_mm2.tile([128, GROUP], FP32, tag="mm2")
                    nc.tensor.transpose(tps[:, 0:128],
                                        outT_sb[:, bass.ts(blk, 128)], eye[:])
                    nc.vector.tensor_copy(out_rows[:, blk, :], tps[:, 0:128])

        # ---- epilogue: out rows were staged per-half; single store ----
        nc.sync.dma_start(
            d_out[:].rearrange("(b p) f -> p b f", p=128), out_rows[:])

    nc.compile()
    return nc


def host_prep(x, r_ij, f_ij, pairwise_mask, neighbors, in2f_w, fw1, fb1, fw2,
              fb2, ssp_mode: str = SSP_MODE):
    """Builds per-core input maps (host-side shard + layout prep)."""
    in_maps = []
    fw2_eff = -fw2 if ssp_mode == "sigmoid_ln" else fw2
    fb2row = np.tile(_bf16(fb2), TPG).reshape(1, GROUP)
    fw1b = _bf16(fw1)
    fw2b = _bf16(fw2_eff)
    w2fb = _bf16(in2f_w)
    fb1c = np.ascontiguousarray(np.asarray(fb1, dtype=np.float32).reshape(F, 1))
    eye = np.eye(128, dtype=np.float32)
    for c in range(NCORES):
        sl = slice(c * BPC, (c + 1) * BPC)
        fij_c = np.asarray(f_ij[sl], dtype=np.float32).reshape(ROWS, G)
        fijT = np.ascontiguousarray(_bf16(fij_c.T))
        r_c = np.ascontiguousarray(
            np.asarray(r_ij[sl], dtype=np.float32).reshape(128, ROWS // 128))
        xT = np.ascontiguousarray(
            _bf16(np.asarray(x[sl], dtype=np.float32).reshape(ATOMS, F).T))
        nbr = np.asarray(neighbors[sl], dtype=np.int64).reshape(BPC, A * N)
        gl = (nbr + (np.arange(BPC, dtype=np.int64) * A)[:, None]).reshape(ROWS)
        # dma_gather idx plane: idx i of gather g2 at [i%16, 64*g2 + i//16], x8
        p16 = gl.astype(np.int16).reshape(ROWS // 1024, 64, 16).transpose(2, 0, 1)
        plane = np.tile(np.ascontiguousarray(p16.reshape(16, ROWS // 16)), (8, 1))
        pm_c = np.ascontiguousarray(
            np.asarray(pairwise_mask[sl], dtype=np.float32).reshape(
                128, ROWS // 128))
        in_maps.append({
            "fijT": fijT, "r_ij": r_c, "xT": xT, "idx": plane,
            "fw1": fw1b, "fw2": fw2b, "in2f_w": w2fb, "fb1": fb1c,
            "eye": eye, "pmask": pm_c, "fb2row": np.ascontiguousarray(fb2row),
        })
    return in_maps


def get_program(fb2_nonzero, need_pmask, ssp_mode=SSP_MODE):
    key = (fb2_nonzero, need_pmask, ssp_mode)
    if key not in _CACHE:
        _CACHE[key] = build_kernel(fb2_nonzero, need_pmask, ssp_mode)
    return _CACHE[key]


def kernel(x, r_ij, f_ij, pairwise_mask, neighbors, in2f_w, fw1, fb1, fw2, fb2,
           _trace=False):
    global LAST_RESULTS
    args = [np.asarray(a) for a in
            (x, r_ij, f_ij, pairwise_mask, neighbors, in2f_w, fw1, fb1, fw2, fb2)]
    x, r_ij, f_ij, pairwise_mask, neighbors, in2f_w, fw1, fb1, fw2, fb2 = args

    fb2_nonzero = bool(np.any(fb2 != 0))
    need_pmask = not bool(np.all(pairwise_mask == 1.0))
    nc = get_program(fb2_nonzero, need_pmask)
    in_maps = host_prep(x, r_ij, f_ij, pairwise_mask, neighbors, in2f_w, fw1,
                        fb1, fw2, fb2)
    try:
        res = run_bass_kernel_spmd(nc, in_maps, core_ids=list(range(NCORES)),
                                   trace=_trace)
    except ModuleNotFoundError:
        # axon client without the NTFF profile hook: retry untraced.
        os.environ["BASS_NEVER_TRACE"] = "1"
        try:
            res = run_bass_kernel_spmd(nc, in_maps,
                                       core_ids=list(range(NCORES)))
        finally:
            os.environ.pop("BASS_NEVER_TRACE", None)
    LAST_RESULTS = res
    out = np.empty((B, A, F), dtype=np.float32)
    for c in range(NCORES):
        out[c * BPC:(c + 1) * BPC] = res.results[c]["out"].reshape(BPC, A, F)
    return out


# revision 26
# speedup vs baseline: 1.0331x; 1.0331x over previous
"""CFConv (SchNet continuous-filter convolution) Bass/Tile kernel for 8x TRN2.

Reference computation (per molecule b):
    W   = ssp(f_ij @ fw1 + fb1) @ fw2 + fb2          (B,A,N,F); ssp = softplus - ln2
    C   = 0.5*(cos(r_ij*pi/5)+1) * (r_ij<5) * mask   (B,A,N)
    y   = x @ in2f_w                                  (B,A,F)
    out = sum_n y[b, nbr[b,a,n], :] * W * C[...,None] (B,A,F)

Sharding: data-parallel over batch B=32 across 8 cores (4 molecules/core).

Per-core device plan (rows = flattened (a,n), 65536 rows in 32 quad-groups of
2048; ACT is the bottleneck engine, ~1 elem/cycle/lane, so ssp costs 2 passes):
  MM1  (PE):  out1T[h,r]  = fw1.T @ f_ijT            4x N=512, feature-major
  ssp  (ACT): t = Exp(out1T + fb1); W1s = Ln(0.5t + 0.5) == softplus(v) - ln2
              (exact; Softplus LUT was overlaid away on this stack, Exp+Ln
              share one resident table set via _pin_act_tables)
  MM2  (PE):  out2[r,f]   = W1s_tile.T @ fw2         16x N=128, row-major
  gath (DMA): y_nbh rows from y_dram (dma_gather, 1024 idxs/instr max)
  mul  (DVE): P = out2_psum * y_nbh                  -> SBUF bf16
  agg  (PE):  outT[f, 2t:2t+2] = P_tile.T @ C_bd     cutoff C folded into C_bd
  epilogue:   PE-transpose outT (per 512-atom half) -> out rows, DMA out.
"""

import os
import sys
from contextlib import ExitStack

import numpy as np

# Prefer /opt/trn_rl_repo (its bass_rust maps pwp "softplus" -> Softplus).
for _p in ("/root/.axon_site/_ro/trn_rl_repo", "/opt/trn_rl_repo"):
    if os.path.isdir(_p) and _p not in sys.path:
        sys.path.insert(0, _p)

import ml_dtypes  # noqa: E402
import concourse.bass as bass  # noqa: E402
import concourse.tile as tile  # noqa: E402
from concourse import bacc, mybir  # noqa: E402
from concourse.bass_utils import run_bass_kernel_spmd  # noqa: E402

BF16 = mybir.dt.bfloat16
FP32 = mybir.dt.float32
I16 = mybir.dt.int16
AF = mybir.ActivationFunctionType
ALU = mybir.AluOpType

B, A, N, G, F = 32, 256, 64, 50, 128
CUTOFF = 5.0
NCORES = 8
BPC = B // NCORES              # molecules per core = 4
ROWS = BPC * A * N             # rows per core = 65536
GROUP = 512                    # rows per group (one PSUM bank)
NGROUPS = ROWS // GROUP        # 128
TPG = GROUP // 128             # 128-row tiles per group = 4
NTILES = ROWS // 128           # 512
ATOMS = BPC * A                # 1024 atoms per core
LN2 = float(np.log(2.0))

# every k-th group uses ACT psum-evac + DVE sbuf-multiply instead of a direct
# DVE psum-source multiply (engine load balance knob; tune on HW). 0 = off.
ACT_ASSIST_EVERY = int(os.environ.get("CF_ACT_ASSIST", "0"))
SSP_MODE = os.environ.get("CF_SSP_MODE", "exp_ln")  # exp_ln|softplus|sigmoid_ln
SUB_ENGINE = os.environ.get("CF_SUB_ENGINE", "pool")  # pool | dve (-ln2 shift)

_CACHE: dict = {}
LAST_RESULTS = None


def _bf16(x):
    return np.asarray(np.asarray(x, dtype=np.float32), dtype=ml_dtypes.bfloat16)


def _pin_act_tables():
    """Restrict the ACT table-set chooser to {natural_log_exp_and_others,
    trig_and_small} so Exp+Ln share one resident LUT set (the default greedy
    chooser alternates exp_and_others/natural_log -> ~83us of table reloads).
    Mutates the functools.cache'd dict in place; set ids stay aligned with
    act_info.json because only the *contents* are blanked."""
    from concourse.hw_specs import get_activation_tables
    keep = {"natural_log_exp_and_others", "trig_and_small"}
    tabs = get_activation_tables("gen3")
    if any(k in tabs for k in keep):
        for k in list(tabs.keys()):
            if k not in keep:
                tabs[k] = set()


def build_kernel(fb2_nonzero: bool, need_pmask: bool, ssp_mode: str = SSP_MODE):
    """Builds the Bass program (shared by all 8 cores)."""
    if ssp_mode == "exp_ln":
        _pin_act_tables()
    nc = bacc.Bacc("TRN2", target_bir_lowering=False, debug=False)

    # ---- DRAM I/O (per-core shards; fw2 is pre-negated in sigmoid_ln mode) ----
    d_fijT = nc.dram_tensor("fijT", [G, ROWS], BF16, kind="ExternalInput")
    d_r = nc.dram_tensor("r_ij", [128, ROWS // 128], FP32, kind="ExternalInput")
    d_xT = nc.dram_tensor("xT", [F, ATOMS], BF16, kind="ExternalInput")
    d_idx = nc.dram_tensor("idx", [128, ROWS // 16], I16, kind="ExternalInput")
    d_fw1 = nc.dram_tensor("fw1", [G, F], BF16, kind="ExternalInput")
    d_fw2 = nc.dram_tensor("fw2", [F, F], BF16, kind="ExternalInput")
    d_w2f = nc.dram_tensor("in2f_w", [F, F], BF16, kind="ExternalInput")
    d_fb1 = nc.dram_tensor("fb1", [F, 1], FP32, kind="ExternalInput")
    d_eye = nc.dram_tensor("eye", [128, 128], FP32, kind="ExternalInput")
    d_pm = nc.dram_tensor("pmask", [128, ROWS // 128], FP32, kind="ExternalInput")
    d_fb2r = nc.dram_tensor("fb2row", [1, GROUP], BF16, kind="ExternalInput")
    d_out = nc.dram_tensor("out", [ATOMS, F], FP32, kind="ExternalOutput")

    with tile.TileContext(nc) as tc, ExitStack() as ctx:
        consts = ctx.enter_context(tc.tile_pool(name="consts", bufs=1))
        w1pool = ctx.enter_context(tc.tile_pool(name="w1", bufs=int(os.environ.get("CF_W1BUFS", "3"))))
        ypool = ctx.enter_context(tc.tile_pool(name="ynbh", bufs=int(os.environ.get("CF_YBUFS", "4"))))
        ppool = ctx.enter_context(tc.tile_pool(name="pmul", bufs=int(os.environ.get("CF_PBUFS", "3"))))
        evpool = ctx.enter_context(tc.tile_pool(name="evac", bufs=3))
        fijpool = ctx.enter_context(tc.tile_pool(name="fij", bufs=int(os.environ.get("CF_FIJBUFS", "4"))))
        outsb = ctx.enter_context(tc.tile_pool(name="outsb", bufs=1))
        ps_mm1 = ctx.enter_context(tc.tile_pool(name="psmm1", bufs=1, space="PSUM"))
        ps_mm2 = ctx.enter_context(tc.tile_pool(name="psmm2", bufs=2, space="PSUM"))
        ps_acc = ctx.enter_context(tc.tile_pool(name="psacc", bufs=2, space="PSUM"))
        dram = ctx.enter_context(tc.tile_pool(name="dram", bufs=1, space="DRAM"))

        # ---- ACT warm-up: a no-dep Sin makes the trig LUT load start at
        # t=0 instead of queueing behind the first input DMAs.
        warm = consts.tile([128, 1], FP32)
        nc.vector.memset(warm[:], 0.0)
        warm2 = consts.tile([128, 1], FP32)
        nc.scalar.activation(warm2[:], warm[:], AF.Sin, bias=warm[:])

        # ---- load constants (r_ij first: it gates the serial C-prologue) ----
        r_sb = consts.tile([128, ROWS // 128], FP32)
        nc.sync.dma_start(r_sb[:], d_r[:])
        fw1 = consts.tile([G, F], BF16)
        nc.sync.dma_start(fw1[:], d_fw1[:])
        # ---- main-loop helpers; MM1(0) is emitted FIRST so the PE starts
        # the quad-0 filter matmul immediately instead of queueing behind the
        # y/C prologue matmuls (saves ~7us of ACT startup stall).
        QG = 4 * GROUP                  # 2048 rows per iteration
        NQG = ROWS // QG                # 32
        FILL = 256                      # tiles per acc-psum fill
        qgpf = FILL // (QG // 128)      # quad-groups per fill = 16
        GHALF = QG // 2                 # 1024 idxs per dma_gather instruction

        def do_mm1(g):
            fij = fijpool.tile([G, QG], BF16, tag="fij")
            nc.sync.dma_start(fij[:], d_fijT[:, bass.ts(g, QG)])
            p1 = ps_mm1.tile([128, QG], FP32, tag="mm1")
            for h in range(4):
                nc.tensor.matmul(p1[:, bass.ts(h, GROUP)], fw1[:],
                                 fij[:, bass.ts(h, GROUP)],
                                 start=True, stop=True)
            return p1

        p1 = do_mm1(0)

        fw2 = consts.tile([F, F], BF16)
        nc.sync.dma_start(fw2[:], d_fw2[:])
        w2f = consts.tile([F, F], BF16)
        nc.sync.dma_start(w2f[:], d_w2f[:])
        fb1 = consts.tile([F, 1], FP32)
        nc.sync.dma_start(fb1[:], d_fb1[:])
        eye = consts.tile([128, 128], FP32)
        nc.sync.dma_start(eye[:], d_eye[:])
        if ssp_mode == "sigmoid_ln":
            fb1n = consts.tile([F, 1], FP32)
            nc.vector.tensor_scalar_mul(fb1n[:], fb1[:], -1.0)
            zerob = consts.tile([128, 1], FP32)
            nc.vector.memset(zerob[:], 0.0)
        if ssp_mode == "exp_ln":
            half = consts.tile([128, 1], FP32)
            nc.vector.memset(half[:], 0.5)
        if fb2_nonzero:
            fb2row = consts.tile([1, GROUP], BF16)
            nc.sync.dma_start(fb2row[:], d_fb2r[:])
            ones1 = consts.tile([1, 128], BF16)
            nc.vector.memset(ones1[:], 1.0)


        # ---- prologue: y = x @ in2f_w  -> y_dram (bf16) ----
        y_dram = dram.tile([ATOMS, F], BF16)
        xT = consts.tile([F, ATOMS], BF16)
        nc.sync.dma_start(xT[:], d_xT[:])
        y_sb = consts.tile([128, ATOMS // 128, F], BF16)
        for blk in range(ATOMS // 128):
            yps = ps_mm2.tile([128, GROUP], FP32, tag="mm2")
            nc.tensor.matmul(yps[:, 0:F], xT[:, bass.ts(blk, 128)], w2f[:],
                             start=True, stop=True)
            nc.vector.tensor_copy(y_sb[:, blk, :], yps[:, 0:F])
        nc.sync.dma_start(
            y_dram[:].rearrange("(b p) f -> p b f", p=128), y_sb[:])

        # ---- prologue: cutoff C -> block-diag C_bd [128, 2*NTILES] bf16 ----
        # cos(t) = sin(pi/2 - t); ACT Sin valid range is [-pi, pi].
        c_nat = consts.tile([128, ROWS // 128], FP32)
        sinb = consts.tile([128, 1], FP32)
        nc.vector.memset(sinb[:], float(np.pi / 2))
        nc.scalar.activation(c_nat[:], r_sb[:], AF.Sin,
                             bias=sinb[:], scale=float(-np.pi / CUTOFF))
        nc.vector.tensor_scalar(c_nat[:], c_nat[:], 0.5, 0.5, ALU.mult, ALU.add)
        rmask = consts.tile([128, ROWS // 128], FP32)
        nc.vector.tensor_scalar(rmask[:], r_sb[:], CUTOFF, None, ALU.is_lt)
        nc.vector.tensor_mul(c_nat[:], c_nat[:], rmask[:])
        if need_pmask:
            pm_sb = consts.tile([128, ROWS // 128], FP32)
            nc.sync.dma_start(pm_sb[:], d_pm[:])
            nc.vector.tensor_mul(c_nat[:], c_nat[:], pm_sb[:])
        # transpose 128-blocks:  c_T[:, 4j+b] = c_nat[:, 128b:128b+128].T[:, j]
        c_T = consts.tile([128, NTILES], FP32)
        nblk = NTILES // 128  # 4
        for b in range(nblk):
            tps = ps_mm2.tile([128, GROUP], FP32, tag="mm2")
            nc.tensor.transpose(tps[:, 0:128], c_nat[:, bass.ts(b, 128)], eye[:])
            nc.vector.tensor_copy(c_T[:, b::nblk], tps[:, 0:128])
        c_bd = consts.tile([128, 2 * NTILES], BF16)
        nc.vector.memset(c_bd[:], 0.0)
        nc.vector.tensor_copy(c_bd[0:64, 0::2], c_T[0:64, :])
        nc.vector.tensor_copy(c_bd[64:128, 1::2], c_T[64:128, :])

        # ---- main loop: quad-groups of 2048 rows (32 iterations) ----
        # mm1 psum is a single 4-bank [128, 2048] tile (bufs=1): ACT fixed
        # overheads amortize over FD=2048. MM1 for iteration k+1 is emitted
        # immediately after Exp(k) (software pipelining) so the PE refills the
        # mm1 bank while ACT runs Ln(k) -- otherwise Exp(k+1) stalls.
        outT_sb = outsb.tile([128, ATOMS], FP32)
        out_rows = outsb.tile([128, ATOMS // 128, F], FP32)
        acc = None

        idxs = consts.tile([128, ROWS // 16], I16)
        nc.sync.dma_start(idxs[:, bass.ts(0, ROWS // 64)],
                          d_idx[:, bass.ts(0, ROWS // 64)])

        def do_gather(g):
            ynbh = ypool.tile([128, QG // 128, F], BF16, tag="ynbh")
            for q in range(2):
                nc.gpsimd.dma_gather(
                    ynbh[:, q * (GHALF // 128):(q + 1) * (GHALF // 128), :],
                    y_dram[:],
                    idxs[:, bass.ts(2 * g + q, GHALF // 16)],
                    GHALF, GHALF, F)
            return ynbh

        ynbh = do_gather(0)
        for g in range(NQG):
            if g % qgpf == 0:
                acc = ps_acc.tile([128, GROUP], FP32, tag="acc")

            # ssp: W1s = softplus(out1T + fb1) - ln2, in SBUF bf16
            w1s = w1pool.tile([128, QG], BF16, tag="w1s")
            if ssp_mode == "exp_ln":
                # t = exp(v + fb1); w1s = ln(0.5*t + 0.5) == ssp(v) exactly.
                ex = w1pool.tile([128, QG], BF16, tag="w1")
                nc.scalar.activation(ex[:], p1[:], AF.Exp, bias=fb1[:])
                if g + 1 < NQG:
                    p1 = do_mm1(g + 1)
                nc.scalar.activation(w1s[:], ex[:], AF.Ln, bias=half[:],
                                     scale=0.5)
            elif ssp_mode == "softplus":
                w1 = w1pool.tile([128, QG], BF16, tag="w1")
                nc.scalar.activation(w1[:], p1[:], AF.Softplus, bias=fb1[:])
                if g + 1 < NQG:
                    p1 = do_mm1(g + 1)
                if SUB_ENGINE == "pool":
                    nc.gpsimd.tensor_scalar_sub(w1s[:], w1[:], LN2)
                else:
                    nc.vector.tensor_scalar_sub(w1s[:], w1[:], LN2)
            else:
                # w1s = ln(2*sigmoid(-(v))) = -(ssp(v)); fw2 pre-negated on host.
                sg = w1pool.tile([128, QG], BF16, tag="w1")
                nc.scalar.activation(sg[:], p1[:], AF.Sigmoid,
                                     bias=fb1n[:], scale=-1.0)
                if g + 1 < NQG:
                    p1 = do_mm1(g + 1)
                nc.scalar.activation(w1s[:], sg[:], AF.Ln, bias=zerob[:],
                                     scale=2.0)

            if 1 <= g <= 3:  # stream remaining idx chunks off the hot path
                nc.sync.dma_start(idxs[:, bass.ts(g, ROWS // 64)],
                                  d_idx[:, bass.ts(g, ROWS // 64)])
            ynbh_cur = ynbh
            if g + 1 < NQG:
                ynbh = do_gather(g + 1)

            # per 512-row half: MM2, multiply, aggregate
            for hh in range(4):
                p2 = ps_mm2.tile([128, GROUP], FP32, tag="mm2")
                for t in range(TPG):
                    nc.tensor.matmul(
                        p2[:, bass.ts(t, F)],
                        w1s[:, bass.ts(hh * TPG + t, 128)], fw2[:],
                        start=True, stop=(not fb2_nonzero))
                if fb2_nonzero:
                    nc.tensor.matmul(p2[:], ones1[:], fb2row[:],
                                     start=False, stop=True,
                                     skip_group_check=True)

                yg = ynbh_cur[:, hh * TPG:(hh + 1) * TPG, :]
                psb = ppool.tile([128, TPG, F], BF16, tag="p")
                if ACT_ASSIST_EVERY and (4 * g + hh) % ACT_ASSIST_EVERY == 0:
                    ev = evpool.tile([128, GROUP], BF16, tag="ev")
                    nc.scalar.copy(ev[:], p2[:])
                    nc.vector.tensor_mul(
                        psb[:].rearrange("p t f -> p (t f)"), ev[:],
                        yg.rearrange("p t f -> p (t f)"))
                else:
                    nc.vector.tensor_mul(
                        psb[:].rearrange("p t f -> p (t f)"), p2[:],
                        yg.rearrange("p t f -> p (t f)"))

                # agg: outT[:, 2tau:2tau+2] = P_tile.T @ C_bd[:, 2tau:2tau+2]
                for t in range(TPG):
                    tau = (4 * g + hh) * TPG + t
                    col = (tau % FILL) * 2
                    nc.tensor.matmul(acc[:, col:col + 2], psb[:, t, :],
                                     c_bd[:, 2 * tau:2 * tau + 2],
                                     start=True, stop=True)

            if g % qgpf == qgpf - 1:
                hseg = g // qgpf
                nc.vector.tensor_copy(outT_sb[:, bass.ts(hseg, GROUP)], acc[:])
                for blk in range(4 * hseg, 4 * hseg + 4):
                    tps = ps_mm2.tile([128, GROUP], FP32, tag="mm2")
                    nc.tensor.transpose(tps[:, 0:128],
                                        outT_sb[:, bass.ts(blk, 128)], eye[:])
                    nc.vector.tensor_copy(out_rows[:, blk, :], tps[:, 0:128])

        # ---- epilogue: out rows were staged per-half; single store ----
        nc.sync.dma_start(
            d_out[:].rearrange("(b p) f -> p b f", p=128), out_rows[:])

    nc.compile()
    return nc


def host_prep(x, r_ij, f_ij, pairwise_mask, neighbors, in2f_w, fw1, fb1, fw2,
              fb2, ssp_mode: str = SSP_MODE):
    """Builds per-core input maps (host-side shard + layout prep)."""
    in_maps = []
    fw2_eff = -fw2 if ssp_mode == "sigmoid_ln" else fw2
    fb2row = np.tile(_bf16(fb2), TPG).reshape(1, GROUP)
    fw1b = _bf16(fw1)
    fw2b = _bf16(fw2_eff)
    w2fb = _bf16(in2f_w)
    fb1c = np.ascontiguousarray(np.asarray(fb1, dtype=np.float32).reshape(F, 1))
    eye = np.eye(128, dtype=np.float32)
    for c in range(NCORES):
        sl = slice(c * BPC, (c + 1) * BPC)
        fij_c = np.asarray(f_ij[sl], dtype=np.float32).reshape(ROWS, G)
        fijT = np.ascontiguousarray(_bf16(fij_c.T))
        r_c = np.ascontiguousarray(
            np.asarray(r_ij[sl], dtype=np.float32).reshape(128, ROWS // 128))
        xT = np.ascontiguousarray(
            _bf16(np.asarray(x[sl], dtype=np.float32).reshape(ATOMS, F).T))
        nbr = np.asarray(neighbors[sl], dtype=np.int64).reshape(BPC, A * N)
        gl = (nbr + (np.arange(BPC, dtype=np.int64) * A)[:, None]).reshape(ROWS)
        # dma_gather idx plane: idx i of gather g2 at [i%16, 64*g2 + i//16], x8
        p16 = gl.astype(np.int16).reshape(ROWS // 1024, 64, 16).transpose(2, 0, 1)
        plane = np.tile(np.ascontiguousarray(p16.reshape(16, ROWS // 16)), (8, 1))
        pm_c = np.ascontiguousarray(
            np.asarray(pairwise_mask[sl], dtype=np.float32).reshape(
                128, ROWS // 128))
        in_maps.append({
            "fijT": fijT, "r_ij": r_c, "xT": xT, "idx": plane,
            "fw1": fw1b, "fw2": fw2b, "in2f_w": w2fb, "fb1": fb1c,
            "eye": eye, "pmask": pm_c, "fb2row": np.ascontiguousarray(fb2row),
        })
    return in_maps


def get_program(fb2_nonzero, need_pmask, ssp_mode=SSP_MODE):
    key = (fb2_nonzero, need_pmask, ssp_mode)
    if key not in _CACHE:
        _CACHE[key] = build_kernel(fb2_nonzero, need_pmask, ssp_mode)
    return _CACHE[key]


def kernel(x, r_ij, f_ij, pairwise_mask, neighbors, in2f_w, fw1, fb1, fw2, fb2,
           _trace=False):
    global LAST_RESULTS
    args = [np.asarray(a) for a in
            (x, r_ij, f_ij, pairwise_mask, neighbors, in2f_w, fw1, fb1, fw2, fb2)]
    x, r_ij, f_ij, pairwise_mask, neighbors, in2f_w, fw1, fb1, fw2, fb2 = args

    fb2_nonzero = bool(np.any(fb2 != 0))
    need_pmask = not bool(np.all(pairwise_mask == 1.0))
    nc = get_program(fb2_nonzero, need_pmask)
    in_maps = host_prep(x, r_ij, f_ij, pairwise_mask, neighbors, in2f_w, fw1,
                        fb1, fw2, fb2)
    try:
        res = run_bass_kernel_spmd(nc, in_maps, core_ids=list(range(NCORES)),
                                   trace=_trace)
    except ModuleNotFoundError:
        # axon client without the NTFF profile hook: retry untraced.
        os.environ["BASS_NEVER_TRACE"] = "1"
        try:
            res = run_bass_kernel_spmd(nc, in_maps,
                                       core_ids=list(range(NCORES)))
        finally:
            os.environ.pop("BASS_NEVER_TRACE", None)
    LAST_RESULTS = res
    out = np.empty((B, A, F), dtype=np.float32)
    for c in range(NCORES):
        out[c * BPC:(c + 1) * BPC] = res.results[c]["out"].reshape(BPC, A, F)
    return out
